# revision 2
# baseline (speedup 1.0000x reference)
"""Minibatch discrimination kernel for 8 Trainium2 NeuronCores.

Math (reference):
    M = einsum('bi,iok->bok', x, T)            # [B, O, K]
    norm[i, j, o] = sum_k |M[i,o,k] - M[j,o,k]|
    out[j, o] = sum_i exp(-norm[i,j,o]) - 1.0  # [B, O]

Strategy:
  - SPMD across 8 cores; core c receives x rotated by -64*c rows and
    transposed on the host: XT = roll(x, -64c).T[:, :320]. Only 320
    b-columns are ever touched per core (windows reach j+256 <= 319).
  - Symmetry: each unordered pair {a, b} is evaluated once. Core c, local
    row j (j = 0..63), covers i in the ring window [j+1, j+256]. Row sums
    (OB) include the full window; the shared d=256 column is counted by
    both partner cores' OB (once for each of its two rows) and therefore
    excluded from the column-sum accumulator (EAC) -> no double count.
  - MT = (x @ T)^T per o-group: [125 = (25 o, 5 k) o-major, 320 b] fp32,
    from 4 fp32 matmuls per group (TT blocks x XT blocks).
  - Per j, per group: dg = |MT - MT[:, j]| in ONE DVE instruction via a
    runtime-registered custom DVE op (ABSOLUTE_DIFF ALU stage) with a
    hand-authored 2x_2PORT perf-mode program (both SBUF read ports, 2
    elem/cycle) for groups 0-2; group 3 uses an ACT Abs activation with
    bias = -MT[:, j] to balance engine load. k-reduce over the 5
    partitions per o via one +BO f16 matmul per group into the 4
    col-strips of the PE array (tile_position=(0,32g)) -> np [128, 256]
    PSUM.
  - One ACT Exp per j: ep = exp(-np) f16, accum_out -> OB column j.
  - Column sums on the PE: eac_ps[:, w0:w0+255] += ID.T @ ep[:, :255]
    (f16 identity matmul, one PSUM accumulation group over all 64 j's).
"""

import numpy as np

import concourse.bacc as bacc
import concourse.mybir as mybir
from concourse.tile import TileContext
from concourse.bass_utils import run_bass_kernel_spmd

B = 512
IN_F = 512
O = 100
K = 5
NCORES = 8
JB = B // NCORES          # 64 output rows per core
NG = 4                    # o-groups
OG = O // NG              # 25 o's per group
PG = OG * K               # 125 partitions per group
W = 256                   # ring window width (d = 1..256)
XB = 320                  # b-columns touched per core (63 + 256 + 1)
EACW = 318                # eac columns 1..318 (d <= 255, j <= 63)
F32 = mybir.dt.float32
F16 = mybir.dt.float16

ASSIGN = "VVVA"           # per-group abs engine: V=DVE custom op, A=ACT
PIPE = 5                  # producer pairs in flight ahead of consumers
WB, NB, EB = 12, 6, 8     # work/psum/exp tile-pool depths


def _build_abs2x_uop():
    """Hand-authored 2x_2PORT DVE uop: two ABSOLUTE_DIFF lanes per cycle,
    mirroring the stock TENSOR_SCALAR slot-18 program. Port0 (SRC_0)
    carries in0[i], port1 (SRC_1) carries in0[i+1]. Elem-i result is
    captured into delay2 at block 2, elem-i+1 into delay3 at block 4;
    writes: WR0_LO <- DELAY_2, WR1_LO <- DELAY_3."""
    from concourse.dve_uop import (
        UopConfig, InpSel, AluInp, DelayInp, OutSel, OutPath, Trigger,
        AluOp, ENABLE,
    )
    u = UopConfig()
    u.enable_input(InpSel.SRC_0, 0)    # lane0 -> block 0 mux PREV_ALU_OUT
    u.enable_input(InpSel.CONST_0, 1)  # lane1 -> delay chain 0
    u.enable_input(InpSel.CONST_1, 2)  # lane2 -> delay chain 1 (unused)
    u.enable_input(InpSel.SRC_1, 3)    # lane3 -> delay chain 2
    d = u.datapath_config
    d[0].enable_alu(AluOp.ABSOLUTE_DIFF, AluInp.PREV_ALU_OUT, AluInp.PREV_DELAY_0)
    d[0].pass_through_delay(0, 1, 2)
    d[1].pass_through_alu()
    d[1].pass_through_delay(0, 1, 2)
    d[2].enable_alu(AluOp.ABSOLUTE_DIFF, AluInp.PREV_DELAY_2, AluInp.PREV_DELAY_0)
    d[2].pass_through_delay(1)
    d[2].enable_delay_from_src(DelayInp.PREV_ALU_OUT, 2)  # capture elem i
    d[3].pass_through_alu()
    d[3].pass_through_delay(2)
    d[4].pass_through_delay(2)
    d[4].enable_delay_from_src(DelayInp.PREV_ALU_OUT, 3)  # capture elem i+1
    for bi in range(5, 8):
        d[bi].pass_through_delay(2, 3)
    u.enable_output(OutSel.DELAY_2, OutPath.WR0_LO)
    u.enable_output(OutSel.DELAY_3, OutPath.WR1_LO)
    u.require_inp0 = ENABLE
    u.require_inp1 = ENABLE
    u.enable_rev_ops = ENABLE
    u.trigger = (Trigger.SRC_TENSOR_DONE, Trigger.NONE, Trigger.NONE)
    u.next_uop = (0, 0, 0)
    return u


def _register_abs_op():
    """Register |in0 - s0| as a custom DVE op at runtime: 1x program from
    the spec compiler, 2x_2PORT program hand-authored. Idempotent."""
    from concourse import dve_ops as D
    from concourse.dve_spec import Spec, Src0, C0, Bin, lower
    from concourse.dve_uop import AluOp, DveOpSpec
    from dataclasses import dataclass

    name = "ABS_SUB2X_MBD"
    for op in D.OPS:
        if op.name == name:
            return op
    spec = Spec(
        body=Bin(AluOp.ABSOLUTE_DIFF, Src0, C0),
        reference=lambda in0, in1, s0, s1, imm2: np.abs(
            in0.astype(np.float32) - s0),
    )

    @dataclass(frozen=True)
    class _AbsOp2x(D.DveOp):
        def compile(self, ver):
            key = (self.name, ver)
            if (r := D._COMPILE_CACHE.get(key)) is not None:
                return r
            uops_1x = lower(self.spec, ver=ver)
            result = DveOpSpec(
                name=self.name,
                opcode=D.get_dve_sub_opcode(self.name),
                uops=uops_1x,
                uops_2x=list(uops_1x),   # unreachable for fp32 input
                uops_2x_2p=[_build_abs2x_uop()],
                perf_max=2,
                rd1_en=False,
            )
            D._COMPILE_CACHE[key] = result
            return result

    row = D._CUSTOM_DVE_ROW_BASE + len(D.OPS)
    assert row < 0x20
    D._SUB_OPCODE_FOR_NAME[name] = row
    op = _AbsOp2x(name, spec, subdim=False, uops_sha={})
    D.OPS.append(op)
    D.CUSTOM_DVE_SPECS[name] = spec
    return op


ABS_OP = _register_abs_op()


def _emit_abs(nc, out, in0, s0):
    """Emit the custom abs op with perf_max=2 (2x_2PORT reachable); the
    generic _custom_dve wrapper hardcodes perf_max=0."""
    import concourse.bass_isa as bass_isa
    from concourse.dve_ops import get_dve_sub_opcode
    from concourse.dve_table_gen import dve_ver_for

    v = nc.vector
    b = v.bass
    op = ABS_OP
    if op.name not in b.m.ant_custom_dve_ops:
        b.m.ant_custom_dve_ops = sorted({*b.m.ant_custom_dve_ops, op.name})
    op.compile(dve_ver_for(b.trn_type))
    shape = bass_isa.CustomDveShape.TTSS
    isa_opcode = b.isa.Opcode[
        f"NEURON_ISA_TPB_OPCODE_CUSTOM_DVE_ANT_{shape.slot()}"].value
    ins = [
        v.lower_ap(in0, for_isa=True, opt=True),
        v.lower_ap(s0, for_isa=True),
        mybir.ImmediateValue(dtype=mybir.dt.float32, value=0.0),
    ]
    outs = [v.lower_ap(out, for_isa=True, opt=True)]
    return v.add_instruction(
        bass_isa.InstCustomDveAnt(
            name=b.get_next_instruction_name(),
            op_name=op.name,
            rd1_en=False,
            subdim=0,
            imm2=0.0,
            shape=shape,
            row=get_dve_sub_opcode(op.name),
            isa_opcode=isa_opcode,
            perf_max=2,
            ins=ins,
            outs=outs,
        )
    )


def _build_nc(hw_loop=0):
    nc = bacc.Bacc()

    xt = nc.declare_dram_parameter("XT", [IN_F, XB], F32, isOutput=False)
    tt = nc.declare_dram_parameter("TT", [IN_F, O * K], F32, isOutput=False)
    bo = nc.declare_dram_parameter("BO", [PG, 32], F16, isOutput=False)
    idm = nc.declare_dram_parameter("IDM", [128, 128], F16, isOutput=False)
    ob = nc.declare_dram_parameter("OB", [128, JB], F32, isOutput=True)
    eac_d = nc.declare_dram_parameter("EAC", [128, EACW], F32, isOutput=True)

    with TileContext(nc) as tc:
        with (
            tc.tile_pool(name="const", bufs=1) as cpool,
            tc.tile_pool(name="work", bufs=WB) as wpool,
            tc.tile_pool(name="mps", bufs=1, space="PSUM") as mpspool,
            tc.tile_pool(name="nps", bufs=NB, space="PSUM") as npspool,
            tc.tile_pool(name="eacps", bufs=1, space="PSUM") as eacpool,
            tc.tile_pool(name="eps", bufs=EB) as epspool,
        ):
            bo_sb = cpool.tile([PG, 32], F16, name="bo_sb")
            nc.sync.dma_start(out=bo_sb[:], in_=bo[:])
            id_sb = cpool.tile([128, 128], F16, name="id_sb")
            nc.sync.dma_start(out=id_sb[:], in_=idm[:])

            t_sb = []
            x_sb = []
            for it in range(4):
                ts = cpool.tile([128, O * K], F32, name=f"t_sb{it}", tag=f"t{it}")
                nc.sync.dma_start(out=ts[:], in_=tt[it * 128:(it + 1) * 128, :])
                t_sb.append(ts)
                xs = cpool.tile([128, XB], F32, name=f"x_sb{it}", tag=f"x{it}")
                nc.sync.dma_start(out=xs[:], in_=xt[it * 128:(it + 1) * 128, :])
                x_sb.append(xs)

            # MT per group: [125 = (o_l, k) o-major, 320 b] fp32
            mt_sb = []
            for g in range(NG):
                mp = mpspool.tile([PG, XB], F32, name="mp", tag="mp")
                for it in range(4):
                    nc.tensor.matmul(
                        mp[:],
                        t_sb[it][:, g * PG:(g + 1) * PG],
                        x_sb[it][:],
                        start=(it == 0),
                        stop=(it == 3),
                    )
                mg = cpool.tile([PG, XB], F32, name=f"mt_sb{g}", tag=f"mt{g}")
                nc.vector.tensor_copy(mg[:], mp[:])
                mt_sb.append(mg)

            # Negated MT columns for the ACT-path Abs bias
            nmt_sb = {}
            for g in range(NG):
                if ASSIGN[g] == "A":
                    ng_t = cpool.tile([PG, XB], F32, name=f"nmt{g}", tag=f"nmt{g}")
                    nc.vector.tensor_scalar(
                        out=ng_t[:], in0=mt_sb[g][:], scalar1=-1.0, scalar2=None,
                        op0=mybir.AluOpType.mult)
                    nmt_sb[g] = ng_t

            ob_sb = cpool.tile([128, JB], F32, name="ob_sb")
            eac_ps = eacpool.tile([128, 512], F32, name="eac_ps", tag="eacps")

            def emit_producers(j):
                w0 = j + 1
                np_t = npspool.tile([128, W], F32, name="np_t", tag="norm")
                for g in range(NG):
                    win = mt_sb[g][:, w0:w0 + W]
                    dg = wpool.tile([PG, W], F16, name="dg", tag="dg")
                    if ASSIGN[g] == "V":
                        _emit_abs(nc, dg[:], win, mt_sb[g][:, j:j + 1])
                    else:
                        nc.scalar.activation(
                            out=dg[:], in_=win,
                            func=mybir.ActivationFunctionType.Abs,
                            bias=nmt_sb[g][:, j:j + 1], scale=1.0)
                    nc.tensor.matmul(
                        np_t[32 * g:32 * g + 32, :], bo_sb[:], dg[:],
                        start=True, stop=True, tile_position=(0, 32 * g),
                        skip_group_check=True)
                return np_t

            def emit_consumer(j, np_t, first, last):
                w0 = j + 1
                ep = epspool.tile([128, W], F16, name="ep", tag="exp")
                nc.scalar.activation(
                    out=ep[:], in_=np_t[:],
                    func=mybir.ActivationFunctionType.Exp,
                    scale=-1.0,
                    accum_out=ob_sb[:, j:j + 1])
                nc.tensor.matmul(
                    eac_ps[:, w0:w0 + W - 1], id_sb[:], ep[:, 0:W - 1],
                    start=first, stop=last, skip_group_check=True)

            import contextlib
            loop_cm = tc.For_i(0, hw_loop, 1) if hw_loop else contextlib.nullcontext()
            with loop_cm:
                pending = []
                for j in range(JB):
                    pending.append((j, emit_producers(j)))
                    if len(pending) > PIPE:
                        jc, npc = pending.pop(0)
                        emit_consumer(jc, npc, first=(jc == 0), last=False)
                for jc, npc in pending:
                    emit_consumer(jc, npc, first=(jc == 0), last=(jc == JB - 1))

            eac_out = cpool.tile([128, EACW], F32, name="eac_out")
            nc.vector.tensor_copy(eac_out[:], eac_ps[:, 1:1 + EACW])
            nc.sync.dma_start(out=eac_d[:], in_=eac_out[:])
            nc.sync.dma_start(out=ob[:], in_=ob_sb[:])

    nc.compile()
    return nc


_NC_CACHE = None


def _get_nc():
    global _NC_CACHE
    if _NC_CACHE is None:
        _NC_CACHE = _build_nc()
    return _NC_CACHE


def _make_consts():
    bo = np.zeros((PG, 32), dtype=np.float16)
    for p in range(PG):
        bo[p, p // K] = 1.0
    idm = np.eye(128, dtype=np.float16)
    return bo, idm


def _in_maps(x, T):
    bo, idm = _make_consts()
    tt = np.ascontiguousarray(np.asarray(T, np.float32).reshape(IN_F, O * K))
    maps = []
    for c in range(NCORES):
        xr = np.roll(np.asarray(x, np.float32), -JB * c, axis=0)
        maps.append({
            "XT": np.ascontiguousarray(xr.T[:, :XB]),
            "TT": tt,
            "BO": bo,
            "IDM": idm,
        })
    return maps


def _assemble(results):
    out = np.zeros((B, O), dtype=np.float64)
    cols = np.arange(EACW) + 1          # local b-coords 1..318
    for c in range(NCORES):
        obc = results[c]["OB"]      # [128, JB] row (j) sums over each window
        eacc = results[c]["EAC"]    # [128, EACW] column (i) sums, local coords
        rows = (cols + JB * c) % B
        for g in range(NG):
            out[JB * c:JB * (c + 1), OG * g:OG * (g + 1)] += \
                obc[32 * g:32 * g + OG, :].T.astype(np.float64)
            out[rows, OG * g:OG * (g + 1)] += \
                eacc[32 * g:32 * g + OG, :].T.astype(np.float64)
    return out.astype(np.float32)


def kernel(x: np.ndarray, T: np.ndarray) -> np.ndarray:
    x = np.ascontiguousarray(np.asarray(x, dtype=np.float32))
    T = np.ascontiguousarray(np.asarray(T, dtype=np.float32))
    assert x.shape == (B, IN_F) and T.shape == (IN_F, O, K)

    nc = _get_nc()
    res = run_bass_kernel_spmd(nc, _in_maps(x, T), list(range(NCORES)))
    return _assemble(res.results)


if __name__ == "__main__":
    rng = np.random.default_rng(0)
    x = rng.standard_normal((B, IN_F), dtype=np.float32)
    T = rng.standard_normal((IN_F, O, K), dtype=np.float32)
    out = kernel(x, T)
    print("out", out.shape, out.dtype, np.abs(out).max())


# revision 3
# speedup vs baseline: 1.4020x; 1.4020x over previous
"""Minibatch discrimination kernel for 8 Trainium2 NeuronCores.

Math (reference):
    M = einsum('bi,iok->bok', x, T)            # [B, O, K]
    norm[i, j, o] = sum_k |M[i,o,k] - M[j,o,k]|
    out[j, o] = sum_i exp(-norm[i,j,o]) - 1.0  # [B, O]

Strategy:
  - SPMD across 8 cores; core c receives x rotated by -64*c rows and
    transposed on the host: XT = roll(x, -64c).T[:, :320]. Only 320
    b-columns are ever touched per core (windows reach j+256 <= 319).
  - Symmetry: each unordered pair {a, b} is evaluated once. Core c, local
    row j (j = 0..63), covers i in the ring window [j+1, j+256]. Row sums
    (OB) include the full window; the shared d=256 column is counted by
    both partner cores' OB (once for each of its two rows) and therefore
    excluded from the column-sum accumulator (EAC) -> no double count.
  - MT = (x @ T)^T per o-group: [125 = (25 o, 5 k) o-major, 320 b] fp32,
    from 4 fp32 matmuls per group (TT blocks x XT blocks).
  - Per j, per group: dg = |MT - MT[:, j]| in ONE DVE instruction via a
    runtime-registered custom DVE op (ABSOLUTE_DIFF ALU stage) with a
    hand-authored 2x_2PORT perf-mode program (both SBUF read ports, 2
    elem/cycle) for groups 0-2; group 3 uses an ACT Abs activation with
    bias = -MT[:, j] to balance engine load. k-reduce over the 5
    partitions per o via one +BO f16 matmul per group into the 4
    col-strips of the PE array (tile_position=(0,32g)) -> np [128, 256]
    PSUM.
  - One ACT Exp per j: ep = exp(-np) f16, accum_out -> OB column j.
  - Column sums on the PE: eac_ps[:, w0:w0+255] += ID.T @ ep[:, :255]
    (f16 identity matmul, one PSUM accumulation group over all 64 j's).
"""

import numpy as np

import concourse.bacc as bacc
import concourse.mybir as mybir
from concourse.tile import TileContext
from concourse.bass_utils import run_bass_kernel_spmd

B = 512
IN_F = 512
O = 100
K = 5
NCORES = 8
JB = B // NCORES          # 64 output rows per core
NG = 4                    # o-groups
OG = O // NG              # 25 o's per group
PG = OG * K               # 125 partitions per group
W = 256                   # ring window width (d = 1..256)
XB = 320                  # b-columns touched per core (63 + 256 + 1)
EACW = 318                # eac columns 1..318 (d <= 255, j <= 63)
F32 = mybir.dt.float32
F16 = mybir.dt.float16

ASSIGN = "VVVA"           # per-group abs engine: V=DVE custom op, A=ACT
PIPE = 5                  # producer pairs in flight ahead of consumers
WB, NB, EB = 12, 6, 8     # work/psum/exp tile-pool depths


def _build_abs2x_uop():
    """Hand-authored 2x_2PORT DVE uop: two ABSOLUTE_DIFF lanes per cycle,
    mirroring the stock TENSOR_SCALAR slot-18 program. Port0 (SRC_0)
    carries in0[i], port1 (SRC_1) carries in0[i+1]. Elem-i result is
    captured into delay2 at block 2, elem-i+1 into delay3 at block 4;
    writes: WR0_LO <- DELAY_2, WR1_LO <- DELAY_3."""
    from concourse.dve_uop import (
        UopConfig, InpSel, AluInp, DelayInp, OutSel, OutPath, Trigger,
        AluOp, ENABLE,
    )
    u = UopConfig()
    u.enable_input(InpSel.SRC_0, 0)    # lane0 -> block 0 mux PREV_ALU_OUT
    u.enable_input(InpSel.CONST_0, 1)  # lane1 -> delay chain 0
    u.enable_input(InpSel.CONST_1, 2)  # lane2 -> delay chain 1 (unused)
    u.enable_input(InpSel.SRC_1, 3)    # lane3 -> delay chain 2
    d = u.datapath_config
    d[0].enable_alu(AluOp.ABSOLUTE_DIFF, AluInp.PREV_ALU_OUT, AluInp.PREV_DELAY_0)
    d[0].pass_through_delay(0, 1, 2)
    d[1].pass_through_alu()
    d[1].pass_through_delay(0, 1, 2)
    d[2].enable_alu(AluOp.ABSOLUTE_DIFF, AluInp.PREV_DELAY_2, AluInp.PREV_DELAY_0)
    d[2].pass_through_delay(1)
    d[2].enable_delay_from_src(DelayInp.PREV_ALU_OUT, 2)  # capture elem i
    d[3].pass_through_alu()
    d[3].pass_through_delay(2)
    d[4].pass_through_delay(2)
    d[4].enable_delay_from_src(DelayInp.PREV_ALU_OUT, 3)  # capture elem i+1
    for bi in range(5, 8):
        d[bi].pass_through_delay(2, 3)
    u.enable_output(OutSel.DELAY_2, OutPath.WR0_LO)
    u.enable_output(OutSel.DELAY_3, OutPath.WR1_LO)
    u.require_inp0 = ENABLE
    u.require_inp1 = ENABLE
    u.enable_rev_ops = ENABLE
    u.trigger = (Trigger.SRC_TENSOR_DONE, Trigger.NONE, Trigger.NONE)
    u.next_uop = (0, 0, 0)
    return u


def _register_abs_op():
    """Register |in0 - s0| as a custom DVE op at runtime: 1x program from
    the spec compiler, 2x_2PORT program hand-authored. Idempotent."""
    from concourse import dve_ops as D
    from concourse.dve_spec import Spec, Src0, C0, Bin, lower
    from concourse.dve_uop import AluOp, DveOpSpec
    from dataclasses import dataclass

    name = "ABS_SUB2X_MBD"
    for op in D.OPS:
        if op.name == name:
            return op
    spec = Spec(
        body=Bin(AluOp.ABSOLUTE_DIFF, Src0, C0),
        reference=lambda in0, in1, s0, s1, imm2: np.abs(
            in0.astype(np.float32) - s0),
    )

    @dataclass(frozen=True)
    class _AbsOp2x(D.DveOp):
        def compile(self, ver):
            key = (self.name, ver)
            if (r := D._COMPILE_CACHE.get(key)) is not None:
                return r
            uops_1x = lower(self.spec, ver=ver)
            result = DveOpSpec(
                name=self.name,
                opcode=D.get_dve_sub_opcode(self.name),
                uops=uops_1x,
                uops_2x=list(uops_1x),   # unreachable for fp32 input
                uops_2x_2p=[_build_abs2x_uop()],
                perf_max=2,
                rd1_en=False,
            )
            D._COMPILE_CACHE[key] = result
            return result

    row = D._CUSTOM_DVE_ROW_BASE + len(D.OPS)
    assert row < 0x20
    D._SUB_OPCODE_FOR_NAME[name] = row
    op = _AbsOp2x(name, spec, subdim=False, uops_sha={})
    D.OPS.append(op)
    D.CUSTOM_DVE_SPECS[name] = spec
    return op


ABS_OP = _register_abs_op()


def _emit_abs(nc, out, in0, s0):
    """Emit the custom abs op with perf_max=2 (2x_2PORT reachable); the
    generic _custom_dve wrapper hardcodes perf_max=0."""
    import concourse.bass_isa as bass_isa
    from concourse.dve_ops import get_dve_sub_opcode
    from concourse.dve_table_gen import dve_ver_for

    v = nc.vector
    b = v.bass
    op = ABS_OP
    if op.name not in b.m.ant_custom_dve_ops:
        b.m.ant_custom_dve_ops = sorted({*b.m.ant_custom_dve_ops, op.name})
    op.compile(dve_ver_for(b.trn_type))
    shape = bass_isa.CustomDveShape.TTSS
    isa_opcode = b.isa.Opcode[
        f"NEURON_ISA_TPB_OPCODE_CUSTOM_DVE_ANT_{shape.slot()}"].value
    ins = [
        v.lower_ap(in0, for_isa=True, opt=True),
        v.lower_ap(s0, for_isa=True),
        mybir.ImmediateValue(dtype=mybir.dt.float32, value=0.0),
    ]
    outs = [v.lower_ap(out, for_isa=True, opt=True)]
    return v.add_instruction(
        bass_isa.InstCustomDveAnt(
            name=b.get_next_instruction_name(),
            op_name=op.name,
            rd1_en=False,
            subdim=0,
            imm2=0.0,
            shape=shape,
            row=get_dve_sub_opcode(op.name),
            isa_opcode=isa_opcode,
            perf_max=2,
            ins=ins,
            outs=outs,
        )
    )


def _build_nc(hw_loop=0):
    nc = bacc.Bacc()

    xt = nc.declare_dram_parameter("XT", [IN_F, XB], F32, isOutput=False)
    tt = nc.declare_dram_parameter("TT", [IN_F, O * K], F32, isOutput=False)
    bo = nc.declare_dram_parameter("BO", [PG, 32], F16, isOutput=False)
    idm = nc.declare_dram_parameter("IDM", [128, 128], F16, isOutput=False)
    ob = nc.declare_dram_parameter("OB", [128, JB], F32, isOutput=True)
    eac_d = nc.declare_dram_parameter("EAC", [128, EACW], F32, isOutput=True)

    with TileContext(nc) as tc:
        with (
            tc.tile_pool(name="const", bufs=1) as cpool,
            tc.tile_pool(name="work", bufs=WB) as wpool,
            tc.tile_pool(name="mps", bufs=1, space="PSUM") as mpspool,
            tc.tile_pool(name="nps", bufs=NB, space="PSUM") as npspool,
            tc.tile_pool(name="eacps", bufs=1, space="PSUM") as eacpool,
            tc.tile_pool(name="eps", bufs=EB) as epspool,
        ):
            bo_sb = cpool.tile([PG, 32], F16, name="bo_sb")
            nc.sync.dma_start(out=bo_sb[:], in_=bo[:])
            id_sb = cpool.tile([128, 128], F16, name="id_sb")
            nc.sync.dma_start(out=id_sb[:], in_=idm[:])

            # Warm the PE's HAM clock gate (cold = 1.2 GHz, warm = 2.4 GHz;
            # ~3.4us of sustained activity un-throttles it) with small
            # matmuls that depend only on the first tiny DMA - they overlap
            # the XT/TT input DMAs, so the MT matmuls start at full clock.
            warm_ps = npspool.tile([128, W], F32, name="np_t", tag="norm")
            for _ in range(44):
                nc.tensor.matmul(
                    warm_ps[0:32, 0:32], bo_sb[:], bo_sb[:],
                    start=True, stop=True, skip_group_check=True)
            nc.vector.tensor_copy(ob_warm := cpool.tile(
                [32, 32], F32, name="warm_sink"), warm_ps[0:32, 0:32])

            t_sb = []
            x_sb = []
            for it in range(4):
                ts = cpool.tile([128, O * K], F32, name=f"t_sb{it}", tag=f"t{it}")
                nc.sync.dma_start(out=ts[:], in_=tt[it * 128:(it + 1) * 128, :])
                t_sb.append(ts)
                xs = cpool.tile([128, XB], F32, name=f"x_sb{it}", tag=f"x{it}")
                nc.sync.dma_start(out=xs[:], in_=xt[it * 128:(it + 1) * 128, :])
                x_sb.append(xs)

            # MT per group: [125 = (o_l, k) o-major, 320 b] fp32
            mt_sb = []
            for g in range(NG):
                mp = mpspool.tile([PG, XB], F32, name="mp", tag="mp")
                for it in range(4):
                    nc.tensor.matmul(
                        mp[:],
                        t_sb[it][:, g * PG:(g + 1) * PG],
                        x_sb[it][:],
                        start=(it == 0),
                        stop=(it == 3),
                    )
                mg = cpool.tile([PG, XB], F32, name=f"mt_sb{g}", tag=f"mt{g}")
                nc.vector.tensor_copy(mg[:], mp[:])
                mt_sb.append(mg)

            # Negated MT columns for the ACT-path Abs bias
            nmt_sb = {}
            for g in range(NG):
                if ASSIGN[g] == "A":
                    ng_t = cpool.tile([PG, XB], F32, name=f"nmt{g}", tag=f"nmt{g}")
                    nc.vector.tensor_scalar(
                        out=ng_t[:], in0=mt_sb[g][:], scalar1=-1.0, scalar2=None,
                        op0=mybir.AluOpType.mult)
                    nmt_sb[g] = ng_t

            ob_sb = cpool.tile([128, JB], F32, name="ob_sb")
            eac_ps = eacpool.tile([128, 512], F32, name="eac_ps", tag="eacps")

            def emit_producers(j):
                w0 = j + 1
                np_t = npspool.tile([128, W], F32, name="np_t", tag="norm")
                for g in range(NG):
                    win = mt_sb[g][:, w0:w0 + W]
                    dg = wpool.tile([PG, W], F16, name="dg", tag="dg")
                    if ASSIGN[g] == "V":
                        _emit_abs(nc, dg[:], win, mt_sb[g][:, j:j + 1])
                    else:
                        nc.scalar.activation(
                            out=dg[:], in_=win,
                            func=mybir.ActivationFunctionType.Abs,
                            bias=nmt_sb[g][:, j:j + 1], scale=1.0)
                    nc.tensor.matmul(
                        np_t[32 * g:32 * g + 32, :], bo_sb[:], dg[:],
                        start=True, stop=True, tile_position=(0, 32 * g),
                        skip_group_check=True)
                return np_t

            def emit_consumer(j, np_t, first, last):
                w0 = j + 1
                ep = epspool.tile([128, W], F16, name="ep", tag="exp")
                nc.scalar.activation(
                    out=ep[:], in_=np_t[:],
                    func=mybir.ActivationFunctionType.Exp,
                    scale=-1.0,
                    accum_out=ob_sb[:, j:j + 1])
                nc.tensor.matmul(
                    eac_ps[:, w0:w0 + W - 1], id_sb[:], ep[:, 0:W - 1],
                    start=first, stop=last, skip_group_check=True)

            import contextlib
            loop_cm = tc.For_i(0, hw_loop, 1) if hw_loop else contextlib.nullcontext()
            with loop_cm:
                pending = []
                for j in range(JB):
                    pending.append((j, emit_producers(j)))
                    if len(pending) > PIPE:
                        jc, npc = pending.pop(0)
                        emit_consumer(jc, npc, first=(jc == 0), last=False)
                for jc, npc in pending:
                    emit_consumer(jc, npc, first=(jc == 0), last=(jc == JB - 1))

            eac_out = cpool.tile([128, EACW], F32, name="eac_out")
            nc.vector.tensor_copy(eac_out[:], eac_ps[:, 1:1 + EACW])
            nc.sync.dma_start(out=eac_d[:], in_=eac_out[:])
            nc.sync.dma_start(out=ob[:], in_=ob_sb[:])

    nc.compile()
    return nc


_NC_CACHE = None


def _get_nc():
    global _NC_CACHE
    if _NC_CACHE is None:
        _NC_CACHE = _build_nc()
    return _NC_CACHE


def _make_consts():
    bo = np.zeros((PG, 32), dtype=np.float16)
    for p in range(PG):
        bo[p, p // K] = 1.0
    idm = np.eye(128, dtype=np.float16)
    return bo, idm


def _in_maps(x, T):
    bo, idm = _make_consts()
    tt = np.ascontiguousarray(np.asarray(T, np.float32).reshape(IN_F, O * K))
    maps = []
    for c in range(NCORES):
        xr = np.roll(np.asarray(x, np.float32), -JB * c, axis=0)
        maps.append({
            "XT": np.ascontiguousarray(xr.T[:, :XB]),
            "TT": tt,
            "BO": bo,
            "IDM": idm,
        })
    return maps


def _assemble(results):
    out = np.zeros((B, O), dtype=np.float64)
    cols = np.arange(EACW) + 1          # local b-coords 1..318
    for c in range(NCORES):
        obc = results[c]["OB"]      # [128, JB] row (j) sums over each window
        eacc = results[c]["EAC"]    # [128, EACW] column (i) sums, local coords
        rows = (cols + JB * c) % B
        for g in range(NG):
            out[JB * c:JB * (c + 1), OG * g:OG * (g + 1)] += \
                obc[32 * g:32 * g + OG, :].T.astype(np.float64)
            out[rows, OG * g:OG * (g + 1)] += \
                eacc[32 * g:32 * g + OG, :].T.astype(np.float64)
    return out.astype(np.float32)


def kernel(x: np.ndarray, T: np.ndarray) -> np.ndarray:
    x = np.ascontiguousarray(np.asarray(x, dtype=np.float32))
    T = np.ascontiguousarray(np.asarray(T, dtype=np.float32))
    assert x.shape == (B, IN_F) and T.shape == (IN_F, O, K)

    nc = _get_nc()
    res = run_bass_kernel_spmd(nc, _in_maps(x, T), list(range(NCORES)))
    return _assemble(res.results)


if __name__ == "__main__":
    rng = np.random.default_rng(0)
    x = rng.standard_normal((B, IN_F), dtype=np.float32)
    T = rng.standard_normal((IN_F, O, K), dtype=np.float32)
    out = kernel(x, T)
    print("out", out.shape, out.dtype, np.abs(out).max())
